# revision 6
# baseline (speedup 1.0000x reference)
"""LoRA QKV parallel linear with per-token slot routing, on 8 TRN2 NeuronCores.

v2.1: like kernel2 (bf16 operands, resident W, transposed-h phase 1,
host-side mask) plus startup-latency tuning:
  - DMA order: A first, then x per-k chunks with W[j=0] interleaved early,
    so the PE can start phase 1 as soon as the first x chunk lands.
  - Phase 1 is k-outer across 6 concurrent PSUM accumulation groups
    (2 token chunks x 3 targets), so each arriving x chunk feeds 6 matmuls
    immediately instead of the whole phase serializing on the last chunk.
  - PSUM: 6 banks phase 1 + 2 banks phase 2; PSUM->SBUF copies alternate
    between Vector and Scalar engines.
"""

import numpy as np
import ml_dtypes

import concourse.bass as bass
import concourse.bacc as bacc
import concourse.mybir as mybir
import concourse.tile as tile

HIDDEN = 2048
Q_SIZE = 2048
KV_SIZE = 512
OUT = Q_SIZE + 2 * KV_SIZE  # 3072
MAX_LORAS = 8
RANK = 16
T = 8192
N_CORES = 8
T_CORE = T // N_CORES  # 1024

P = 128
NT = T_CORE // P          # 8 token tiles per core
KC = HIDDEN // P          # 16 k-chunks
OJ = OUT // 512           # 6 output chunks of 512
GR = MAX_LORAS * RANK     # 128 = all slots*ranks for one target group
F32 = mybir.dt.float32
BF16 = mybir.dt.bfloat16
BF = ml_dtypes.bfloat16

_NC_CACHE = {}


def build_nc(reps=1, timing=False):
    """Build the SPMD Bass program (same program on every core).

    reps>1 repeats the whole body; timing=True makes all inputs Internal
    (garbage contents, nothing to upload) for wall-clock slope timing."""
    nc = bacc.Bacc("TRN2", target_bir_lowering=False, debug=False, num_devices=N_CORES)

    kin = "Internal" if timing else "ExternalInput"
    xprep = nc.dram_tensor("xprep", [P, KC * T_CORE], BF16, kind=kin).ap()
    wprep = nc.dram_tensor("wprep", [P, OJ * KC * 512], BF16, kind=kin).ap()
    aprep = nc.dram_tensor("aprep", [P, KC * 3 * GR], BF16, kind=kin).ap()
    bprep = nc.dram_tensor("bprep", [P, OUT], BF16, kind=kin).ap()
    mprep = nc.dram_tensor("mprep", [P, T_CORE], F32, kind=kin).ap()
    y = nc.dram_tensor("y", [T_CORE, OUT], F32, kind="ExternalOutput").ap()

    with tile.TileContext(nc) as tc:
      for _rep in range(reps):
        with (
            tc.tile_pool(name="xsb", bufs=1) as xpool,
            tc.tile_pool(name="asb", bufs=1) as apool,
            tc.tile_pool(name="bsb", bufs=1) as bpool,
            tc.tile_pool(name="msb", bufs=1) as mpool,
            tc.tile_pool(name="wsb", bufs=1) as wpool,
            tc.tile_pool(name="hm", bufs=1) as hpool,
            tc.tile_pool(name="o", bufs=4) as opool,
        ):
            # --- resident inputs; DMA issue order tuned for PE start ---
            asb = apool.tile([P, KC * 3 * GR], BF16)    # free = k*384 + g*128+l*16+r
            nc.sync.dma_start(asb[:, 0:2 * 384], aprep[:, 0:2 * 384])
            xsb = xpool.tile([P, KC * T_CORE], BF16)    # free = k*1024 + t
            wsb = wpool.tile([P, OJ * KC * 512], BF16)  # free = j*8192 + k*512 + o
            nc.sync.dma_start(xsb[:, 0:T_CORE], xprep[:, 0:T_CORE])
            nc.sync.dma_start(asb[:, 2 * 384:KC * 384], aprep[:, 2 * 384:KC * 384])
            for k in range(1, KC):
                nc.sync.dma_start(xsb[:, k * T_CORE:(k + 1) * T_CORE],
                                  xprep[:, k * T_CORE:(k + 1) * T_CORE])
            msb = mpool.tile([P, T_CORE], F32)          # maskT, free = t
            nc.sync.dma_start(msb[:], mprep[:, :])
            bsb = bpool.tile([P, OUT], BF16)            # free = o (q|k|v packed)
            nc.sync.dma_start(bsb[:], bprep[:, :])
            for j in range(OJ):
                nc.sync.dma_start(wsb[:, j * 8192:(j + 1) * 8192],
                                  wprep[:, j * 8192:(j + 1) * 8192])

            # --- phase 1: hmT[lr, t], k-outer across 6 PSUM groups ---
            hmT = hpool.tile([P, 2 * 3 * 512], BF16)    # free = (c*3+g)*512 + t
            with tc.tile_pool(name="hps", bufs=1, space="PSUM") as hpsum:
                hps = [hpsum.tile([P, 512], F32, name=f"hps{n}") for n in range(6)]
                for k in range(KC):
                    for c in range(2):
                        for g in range(3):
                            nc.tensor.matmul(
                                hps[c * 3 + g][:],
                                lhsT=asb[:, k * 384 + g * GR: k * 384 + (g + 1) * GR],
                                rhs=xsb[:, k * T_CORE + c * 512:
                                        k * T_CORE + (c + 1) * 512],
                                start=(k == 0), stop=(k == KC - 1))
                for c in range(2):
                    for g in range(3):
                        nc.vector.tensor_tensor(
                            hmT[:, (c * 3 + g) * 512:(c * 3 + g + 1) * 512],
                            hps[c * 3 + g][:], msb[:, c * 512:(c + 1) * 512],
                            op=mybir.AluOpType.mult)

            # --- phase 2: fused base + LoRA GEMM per 128x512 output tile ---
            with tc.tile_pool(name="ops", bufs=5, space="PSUM") as opsum:
                for j in range(OJ):
                    g = 0 if j < 4 else (1 if j == 4 else 2)
                    for i in range(NT):
                        c, q = divmod(i, 4)
                        ops = opsum.tile([P, 512], F32)
                        for k in range(KC):
                            nc.tensor.matmul(
                                ops[:],
                                lhsT=xsb[:, k * T_CORE + i * P:
                                         k * T_CORE + (i + 1) * P],
                                rhs=wsb[:, j * 8192 + k * 512:
                                        j * 8192 + (k + 1) * 512],
                                start=(k == 0), stop=False)
                        nc.tensor.matmul(
                            ops[:],
                            lhsT=hmT[:, (c * 3 + g) * 512 + q * P:
                                     (c * 3 + g) * 512 + (q + 1) * P],
                            rhs=bsb[:, j * 512:(j + 1) * 512],
                            start=False, stop=True)
                        osb = opool.tile([P, 512], F32)
                        nc.vector.tensor_copy(osb[:], ops[:])
                        nc.sync.dma_start(
                            y[i * P:(i + 1) * P, j * 512:(j + 1) * 512], osb[:])
    nc.compile()
    return nc


def prep_in_maps(x, weight, lora_A, lora_B_q, lora_B_k, lora_B_v,
                 lora_scaling, token_to_slot):
    x = np.asarray(x, dtype=np.float32)
    weight = np.asarray(weight, dtype=np.float32)
    lora_A = np.asarray(lora_A, dtype=np.float32)
    lora_B_q = np.asarray(lora_B_q, dtype=np.float32)
    lora_B_k = np.asarray(lora_B_k, dtype=np.float32)
    lora_B_v = np.asarray(lora_B_v, dtype=np.float32)
    lora_scaling = np.asarray(lora_scaling, dtype=np.float32)
    slot = np.asarray(token_to_slot)

    # wprep[p, j*8192 + kh*512 + o] = W[j*512+o, kh*128+p]
    wprep = np.ascontiguousarray(
        weight.T.reshape(KC, P, OJ, 512).transpose(1, 2, 0, 3).reshape(P, OJ * KC * 512)
    ).astype(BF)
    # aprep[p, kh*384 + g*128+l*16+r] = A[l, g, r, kh*128+p]
    a2 = lora_A.transpose(1, 0, 2, 3).reshape(3 * GR, HIDDEN)
    aprep = np.ascontiguousarray(
        a2.T.reshape(KC, P, 3 * GR).transpose(1, 0, 2).reshape(P, KC * 3 * GR)
    ).astype(BF)
    # bprep[l*16+r, o] packed [bq | bk | bv], no scaling (folded into mask)
    bq2 = lora_B_q.transpose(0, 2, 1).reshape(GR, Q_SIZE)
    bk2 = lora_B_k.transpose(0, 2, 1).reshape(GR, KV_SIZE)
    bv2 = lora_B_v.transpose(0, 2, 1).reshape(GR, KV_SIZE)
    bprep = np.ascontiguousarray(
        np.concatenate([bq2, bk2, bv2], axis=1)).astype(BF)
    # maskT[l*16+r, t] = scaling[l] * (slot[t]==l)
    m8 = (slot[None, :] == np.arange(MAX_LORAS, dtype=slot.dtype)[:, None])
    m8 = m8.astype(np.float32) * lora_scaling[:, None]
    mfull = np.repeat(m8, RANK, axis=0)  # (128, T)

    xT = x.T  # (HIDDEN, T)
    in_maps = []
    for cix in range(N_CORES):
        xc = xT[:, cix * T_CORE:(cix + 1) * T_CORE]  # (2048, 1024)
        xprep = np.ascontiguousarray(
            xc.reshape(KC, P, T_CORE).transpose(1, 0, 2).reshape(P, KC * T_CORE)
        ).astype(BF)
        in_maps.append({
            "xprep": xprep,
            "wprep": wprep,
            "aprep": aprep,
            "bprep": bprep,
            "mprep": np.ascontiguousarray(
                mfull[:, cix * T_CORE:(cix + 1) * T_CORE]),
        })
    return in_maps


def kernel(**inputs):
    from concourse.bass_utils import run_bass_kernel_spmd
    if "nc" not in _NC_CACHE:
        _NC_CACHE["nc"] = build_nc()
    nc = _NC_CACHE["nc"]
    in_maps = prep_in_maps(**inputs)
    res = run_bass_kernel_spmd(nc, in_maps, core_ids=list(range(N_CORES)))
    return np.concatenate([r["y"] for r in res.results], axis=0)
